# revision 1
# baseline (speedup 1.0000x reference)
# Trainium2 Bass kernel for LinearAttention (nn_LinearAttention_87686052315975).
#
# Reference computation (per batch element b of 16):
#   xf = x[b].reshape(512, 4096)                      # [c, l]
#   qkv = w_qkv @ xf                                  # [1536, l]
#   q, k, v split into 8 heads x 64 dims
#   k = softmax(k, axis=l)
#   context_h = k_h @ v_h^T                           # [64, 64]
#   out_h = context_h^T @ q_h                         # [64, l]
#   y = w_out @ concat(out_h) + b_out                 # [512, l]
#
# Sharding: data-parallel over batch. 16 batches / 8 cores = 2 per core.
# No collectives needed; each core produces its own output slice.
#
# Per-core kernel structure (per batch, l chunked by 512):
#   Pass A: q = w_q^T-form matmul (kept resident in SBUF, [512, 4096]);
#           kT/vT computed transposed (l on partitions) so the context
#           contraction over l maps onto the PE K dim;
#           E = exp(kT) (no max subtraction needed: |k| ~ N(0,1));
#           ctx_h[d, e] += E_h^T-contract-vT_h via matmul, with a ones
#           column appended to vT so column 64 accumulates rowsum(E).
#   Finalize: ctx_n = ctx * (1/s) per row; pack head pairs into a
#           block-diagonal [128, 128] lhsT via SBUF->SBUF DMA.
#   Pass B: out = ctxP^T-contract-q (one matmul per head pair);
#           y = w_out^T-form matmul + bias; DMA out.
#
# All big matmuls run as float32r (split-precision fp32, 1 cycle/row at
# N>=256 vs 4 for plain fp32). The small context matmuls (N=65) run at
# 4 cycles/row regardless; dtype for them is configurable.

import os
import numpy as np
from contextlib import ExitStack

import concourse.bass as bass
import concourse.bacc as bacc
import concourse.mybir as mybir
import concourse.tile as tile

# ---- problem constants (hardcoded per contract) ----
B, DIM, HGT, WID = 16, 512, 64, 64
L = HGT * WID            # 4096
HEADS, DH = 8, 64
HIDDEN = HEADS * DH      # 512
NCORES = 8
BPC = B // NCORES        # 2 batches per core
P = 128
CHUNK = 512
NCHUNK = L // CHUNK      # 8
KT = DIM // P            # 4 contraction tiles over channels
MT = DIM // P            # 4 output row tiles
LM = CHUNK // P          # 4 l-subtiles per chunk
NPAIR = HEADS // 2       # 4 head pairs
VW = DH + 2              # per-head vT width: 64 v cols + 2 ones cols (even N)

F32 = mybir.dt.float32
F32R = mybir.dt.float32r
MM_DT = mybir.dt.float32r     # dtype for the big (N=512) matmuls
CTX_DT = mybir.dt.float32r    # dtype for the small context matmuls


def _mm(ap, dt):
    return ap.bitcast(dt)


def build_kernel(ctx: ExitStack, tc: "tile.TileContext", x_in, wqkvT_in, woutT_in,
                 bias_in, y_out):
    nc = tc.nc

    wpool = ctx.enter_context(tc.tile_pool(name="weights", bufs=1))
    qpool = ctx.enter_context(tc.tile_pool(name="qres", bufs=1))
    xpool = ctx.enter_context(tc.tile_pool(name="xc", bufs=8))
    epool = ctx.enter_context(tc.tile_pool(name="ev", bufs=8))
    opool = ctx.enter_context(tc.tile_pool(name="osb", bufs=8))
    ypool = ctx.enter_context(tc.tile_pool(name="ysb", bufs=3))
    cpool = ctx.enter_context(tc.tile_pool(name="ctxacc", bufs=1))
    ppool = ctx.enter_context(tc.tile_pool(name="ctxp", bufs=2))
    psmm = ctx.enter_context(tc.tile_pool(name="psmm", bufs=4, space="PSUM"))
    psctx = ctx.enter_context(tc.tile_pool(name="psctx", bufs=2, space="PSUM"))

    # ---- load weights once ----
    wqkv_sb = []
    for k in range(KT):
        t = wpool.tile([P, 3 * HIDDEN], F32R, tag=f"wqkv{k}", name=f"wqkv{k}")
        nc.sync.dma_start(t[:], wqkvT_in[k * P:(k + 1) * P, :])
        wqkv_sb.append(t)
    wout_sb = []
    for k in range(KT):
        t = wpool.tile([P, DIM], F32R, tag=f"wout{k}", name=f"wout{k}")
        nc.sync.dma_start(t[:], woutT_in[k * P:(k + 1) * P, :])
        wout_sb.append(t)
    bias_sb = wpool.tile([P, MT], F32, tag="bias", name="bias")
    nc.sync.dma_start(bias_sb[:], bias_in[:])

    for b in range(BPC):
        # persistent q for this batch: 4 tiles [128, 4096]
        q_sb = [qpool.tile([P, L], F32R, tag=f"q{m}", name=f"q{m}") for m in range(MT)]
        # per-pair context accumulators [128, 132]
        ctx_acc = [cpool.tile([P, 2 * VW], F32, tag=f"ctxacc{p}", name=f"ctxacc{p}")
                   for p in range(NPAIR)]
        # block-diagonal lhsT tiles for pass B; zero-filled early so the
        # finalize chain stays short (HAM stays warm between passes)
        ctxP = []
        for p in range(NPAIR):
            t = ppool.tile([P, P], F32R, tag=f"p{p}", name=f"p{p}")
            nc.vector.tensor_scalar(t[:], wout_sb[0][:, 0:P], 0.0, None,
                                    mybir.AluOpType.mult)
            ctxP.append(t)

        # ---------------- Pass A ----------------
        for i in range(NCHUNK):
            ls = slice(i * CHUNK, (i + 1) * CHUNK)
            xc = []
            for k in range(KT):
                t = xpool.tile([P, CHUNK], F32R, tag="xc", name="xc")
                nc.sync.dma_start(t[:], x_in[b, k * P:(k + 1) * P, ls])
                xc.append(t)

            # q projection: q[o, l] for o-tile m
            for m in range(MT):
                ps = psmm.tile([P, CHUNK], F32, tag="mm", name="mm")
                for k in range(KT):
                    nc.tensor.matmul(
                        ps[:],
                        _mm(wqkv_sb[k][:, m * P:(m + 1) * P], MM_DT),
                        _mm(xc[k][:], MM_DT),
                        start=(k == 0), stop=(k == KT - 1))
                nc.vector.tensor_copy(q_sb[m][:, ls], ps[:])

            # kT/vT projection (l on partitions), exp, ones-append
            E_t, vT_t = [], []
            for lm in range(LM):
                # k half -> E = exp(kT)
                ps = psmm.tile([P, CHUNK], F32, tag="mm", name="mm")
                for k in range(KT):
                    nc.tensor.matmul(
                        ps[:],
                        _mm(xc[k][:, lm * P:(lm + 1) * P], MM_DT),
                        _mm(wqkv_sb[k][:, HIDDEN:2 * HIDDEN], MM_DT),
                        start=(k == 0), stop=(k == KT - 1))
                e = epool.tile([P, CHUNK], F32R, tag="E", name="E")
                nc.scalar.activation(e[:], ps[:],
                                     mybir.ActivationFunctionType.Exp)
                E_t.append(e)

                # v half -> vT with a ones column per head ([128, 8*65])
                ps = psmm.tile([P, CHUNK], F32, tag="mm", name="mm")
                for k in range(KT):
                    nc.tensor.matmul(
                        ps[:],
                        _mm(xc[k][:, lm * P:(lm + 1) * P], MM_DT),
                        _mm(wqkv_sb[k][:, 2 * HIDDEN:3 * HIDDEN], MM_DT),
                        start=(k == 0), stop=(k == KT - 1))
                v = epool.tile([P, HEADS * VW], F32R, tag="vT", name="vT")
                v_view = v[:].rearrange("p (h e) -> p h e", e=VW)
                nc.vector.tensor_copy(
                    v_view[:, :, 0:DH],
                    ps[:].rearrange("p (h e) -> p h e", e=DH))
                nc.vector.tensor_scalar(
                    v_view[:, :, DH:DH + 2],
                    ps[:].rearrange("p (h e) -> p h e", e=DH)[:, :, 0:2],
                    0.0, 1.0, mybir.AluOpType.mult, mybir.AluOpType.add)
                vT_t.append(v)

            # context accumulation, one matmul per head PAIR:
            # out[0:64, 0:66] = ctx_h0 (+rowsum col 64),
            # out[64:128, 66:132] = ctx_h1 (+rowsum col 130);
            # off-diagonal blocks are computed but never read.
            for p in range(NPAIR):
                pc = psctx.tile([P, 2 * VW], F32, tag="ctx", name="ctx")
                for lm in range(LM):
                    nc.tensor.matmul(
                        pc[:],
                        _mm(E_t[lm][:, p * P:(p + 1) * P], CTX_DT),
                        _mm(vT_t[lm][:, p * 2 * VW:(p + 1) * 2 * VW], CTX_DT),
                        start=(lm == 0), stop=(lm == LM - 1))
                if i == 0:
                    nc.vector.tensor_copy(ctx_acc[p][:], pc[:])
                else:
                    nc.vector.tensor_add(ctx_acc[p][:], ctx_acc[p][:], pc[:])

        # ---------------- Finalize: normalize into block-diag ctxP -------
        for p in range(NPAIR):
            acc = ctx_acc[p]
            nc.vector.reciprocal(acc[0:DH, DH:DH + 1], acc[0:DH, DH:DH + 1])
            nc.vector.reciprocal(acc[DH:P, 2 * VW - 2:2 * VW - 1],
                                 acc[DH:P, 2 * VW - 2:2 * VW - 1])
            nc.vector.tensor_scalar_mul(ctxP[p][0:DH, 0:DH],
                                        acc[0:DH, 0:DH],
                                        acc[0:DH, DH:DH + 1])
            nc.vector.tensor_scalar_mul(ctxP[p][DH:P, DH:P],
                                        acc[DH:P, VW:VW + DH],
                                        acc[DH:P, 2 * VW - 2:2 * VW - 1])

        # ---------------- Pass B ----------------
        for i in range(NCHUNK):
            ls = slice(i * CHUNK, (i + 1) * CHUNK)
            out_sb = []
            for p in range(NPAIR):
                ps = psmm.tile([P, CHUNK], F32, tag="mm", name="mm")
                nc.tensor.matmul(ps[:], _mm(ctxP[p][:], MM_DT),
                                 _mm(q_sb[p][:, ls], MM_DT),
                                 start=True, stop=True)
                o = opool.tile([P, CHUNK], F32R, tag="osb", name="osb")
                nc.scalar.copy(o[:], ps[:])
                out_sb.append(o)
            for m in range(MT):
                ps = psmm.tile([P, CHUNK], F32, tag="mm", name="mm")
                for k in range(KT):
                    nc.tensor.matmul(
                        ps[:],
                        _mm(wout_sb[k][:, m * P:(m + 1) * P], MM_DT),
                        _mm(out_sb[k][:], MM_DT),
                        start=(k == 0), stop=(k == KT - 1))
                y = ypool.tile([P, CHUNK], F32, tag="ysb", name="ysb")
                nc.vector.tensor_scalar_add(y[:], ps[:],
                                            bias_sb[:, m:m + 1])
                nc.sync.dma_start(y_out[b, m * P:(m + 1) * P, ls], y[:])


def build_module():
    nc = bacc.Bacc("TRN2", target_bir_lowering=False, debug=False,
                   num_devices=NCORES)
    x_in = nc.dram_tensor("x", [BPC, DIM, L], F32R, kind="ExternalInput")
    wqkvT_in = nc.dram_tensor("w_qkvT", [DIM, 3 * HIDDEN], F32R,
                              kind="ExternalInput")
    woutT_in = nc.dram_tensor("w_outT", [HIDDEN, DIM], F32R,
                              kind="ExternalInput")
    bias_in = nc.dram_tensor("bias", [P, MT], F32, kind="ExternalInput")
    y_out = nc.dram_tensor("y", [BPC, DIM, L], F32, kind="ExternalOutput")
    with tile.TileContext(nc) as tc:
        with ExitStack() as ctx:
            build_kernel(ctx, tc, x_in, wqkvT_in, woutT_in, bias_in, y_out)
    nc.compile()
    return nc


def make_in_maps(x, w_qkv, w_out, b_out):
    x = np.ascontiguousarray(x, dtype=np.float32).reshape(B, DIM, L)
    wqkvT = np.ascontiguousarray(np.asarray(w_qkv, dtype=np.float32).T)
    woutT = np.ascontiguousarray(np.asarray(w_out, dtype=np.float32).T)
    bias = np.ascontiguousarray(
        np.asarray(b_out, dtype=np.float32).reshape(MT, P).T)
    in_maps = []
    for c in range(NCORES):
        in_maps.append({
            "x": x[c * BPC:(c + 1) * BPC],
            "w_qkvT": wqkvT,
            "w_outT": woutT,
            "bias": bias,
        })
    return in_maps


_NC_CACHE = None


def kernel(x, w_qkv, w_out, b_out, *, trace=False, trace_kwargs=None):
    """Full inputs in, full output out. Shards batch across 8 NeuronCores."""
    global _NC_CACHE
    from concourse.bass_utils import run_bass_kernel_spmd

    if _NC_CACHE is None:
        _NC_CACHE = build_module()
    nc = _NC_CACHE

    in_maps = make_in_maps(x, w_qkv, w_out, b_out)
    kw = dict(trace_kwargs or {})
    res = run_bass_kernel_spmd(nc, in_maps, list(range(NCORES)),
                               trace=trace, **kw)
    y = np.empty((B, DIM, HGT, WID), dtype=np.float32)
    for c in range(NCORES):
        y[c * BPC:(c + 1) * BPC] = res.results[c]["y"].reshape(
            BPC, DIM, HGT, WID)
    kernel.last_results = res
    return y



# revision 11
# speedup vs baseline: 1.4141x; 1.4141x over previous
# Trainium2 Bass kernel for LinearAttention (nn_LinearAttention_87686052315975).
#
# Reference computation (per batch element b of 16):
#   xf = x[b].reshape(512, 4096)                      # [c, l]
#   qkv = w_qkv @ xf; q, k, v split into 8 heads x 64 dims
#   k = softmax(k, axis=l)
#   context_h = k_h @ v_h^T                           # [64, 64]
#   out_h = context_h^T @ q_h                          # [64, l]
#   y = w_out @ concat(out_h) + b_out                 # [512, l]
#
# Key restructure vs a direct mapping: since context_h is tiny, fold it into
# the weights.  y = sum_h Wout_h ctxn_h^T Wq_h x = M x with M [512, 512]
# depending only on ctx (data-dependent) and the fixed weights.  This removes
# the q projection (q never materialized) and the per-l attention pass;
# after the k/v sweep we build M (~10k PE cycles) and do one plain matmul
# y = M x + bias.
#
# Per-batch structure (2 batches per core, data-parallel over 8 cores):
#   Pass 1 (l chunked by 512):  kT/vT computed transposed (l on partitions)
#     so the context contraction over l maps onto the PE K dim; E = exp(kT)
#     cast to bf16; vT cast to bf16 with a ones column per head appended so
#     the context matmul also accumulates rowsum(E) (softmax denominator).
#     ctx accumulates in PSUM across all 32 l-subtiles (2 head-pairs per
#     bank, block-diagonal packing).  bf16 runs the N=132 context matmuls at
#     1 cycle/row (fp32r would pay 4x at N<256).  x is also cast to a
#     resident bf16 copy for pass 2.
#   Finalize:  ctxn = ctx * (1/rowsum) into block-diag bf16 tiles.
#   Build M:   A_pair = ctxn_pair^T-contract-Wq_pair  [128, 512]
#              M^T[c, o] = sum_pairs A_pair^T-contract-WoutT_pair
#   Pass 2:    y = (M^T)^T-contract-x_bf16 + bias; DMA out.
#
# Big fp32 matmuls (k/v projection) run as float32r (1 cycle/row at N>=512).
# Everything downstream of exp runs bf16 (inputs only; PSUM accumulation is
# fp32) — well inside the 2e-2 tolerance.

import numpy as np
from contextlib import ExitStack

import concourse.bass as bass
import concourse.bacc as bacc
import concourse.mybir as mybir
import concourse.tile as tile

# ---- problem constants (hardcoded per contract) ----
B, DIM, HGT, WID = 16, 512, 64, 64
L = HGT * WID            # 4096
HEADS, DH = 8, 64
HIDDEN = HEADS * DH      # 512
NCORES = 8
BPC = B // NCORES        # 2 batches per core
P = 128
CHUNK = 512
NCHUNK = L // CHUNK      # 8
KT = DIM // P            # 4 contraction tiles over channels
MT = DIM // P            # 4 output row tiles
LM = CHUNK // P          # 4 l-subtiles per chunk
NPAIR = HEADS // 2       # 4 head pairs
VW = DH + 2              # per-head vT width: 64 v cols + 2 ones cols (even N)
CTXW = 2 * VW            # 132: one pair's context block width

F32 = mybir.dt.float32
F32R = mybir.dt.float32r
BF16 = mybir.dt.bfloat16


def _f32(ap):
    return ap.bitcast(F32)


def build_kernel(ctx: ExitStack, tc: "tile.TileContext", x_in, wkvT_in, wq_in,
                 woutT_in, bias_in, y_out):
    nc = tc.nc

    wpool = ctx.enter_context(tc.tile_pool(name="weights", bufs=1))
    xpool = ctx.enter_context(tc.tile_pool(name="xc", bufs=8))
    xbpool = ctx.enter_context(tc.tile_pool(name="xbf", bufs=8))
    epool = ctx.enter_context(tc.tile_pool(name="ev", bufs=8))
    cpool = ctx.enter_context(tc.tile_pool(name="ctxp", bufs=8))
    apool = ctx.enter_context(tc.tile_pool(name="absf", bufs=4))
    mpool = ctx.enter_context(tc.tile_pool(name="mtbf", bufs=8))
    rpool = ctx.enter_context(tc.tile_pool(name="recip", bufs=8))
    ypool = ctx.enter_context(tc.tile_pool(name="ysb", bufs=4))
    psmm = ctx.enter_context(tc.tile_pool(name="psmm", bufs=4, space="PSUM"))
    psctx = ctx.enter_context(tc.tile_pool(name="psctx", bufs=4, space="PSUM"))

    # ---- load weights once ----
    # k/v projection weights, transposed form [c, 1024], kept fp32(r)
    wkv_sb = []
    for k in range(KT):
        t = wpool.tile([P, 2 * HIDDEN], F32R, tag=f"wkv{k}", name=f"wkv{k}")
        nc.sync.dma_start(t[:], wkvT_in[k * P:(k + 1) * P, :])
        wkv_sb.append(t)
    # Wq rows [(h,d), c] and WoutT rows [(h,e), o], cast to bf16
    wq_bf, wout_bf = [], []
    for k in range(KT):
        s = xpool.tile([P, DIM], F32, tag="stg", name="wq_stage")
        nc.sync.dma_start(s[:], wq_in[k * P:(k + 1) * P, :])
        t = wpool.tile([P, DIM], BF16, tag=f"wq{k}", name=f"wq{k}")
        nc.vector.tensor_copy(t[:], s[:])
        wq_bf.append(t)
    for k in range(KT):
        s = xpool.tile([P, DIM], F32, tag="stg", name="wout_stage")
        nc.sync.dma_start(s[:], woutT_in[k * P:(k + 1) * P, :])
        t = wpool.tile([P, DIM], BF16, tag=f"wout{k}", name=f"wout{k}")
        nc.vector.tensor_copy(t[:], s[:])
        wout_bf.append(t)
    bias_sb = wpool.tile([P, MT], F32, tag="bias", name="bias")
    nc.sync.dma_start(bias_sb[:], bias_in[:])

    x_bf = {}      # batch -> 4 resident bf16 tiles [128, 4096]
    ctxP = {}      # batch -> 4 block-diag bf16 [128, 128] normalized ctx
    ctx_ps = {}    # batch -> 2 PSUM tiles [128, 264] (2 pairs each)

    def pass1(b):
        x_bf[b] = [xbpool.tile([P, L], BF16, tag="xbf", name=f"xbf{b}_{k}")
                   for k in range(KT)]
        ctx_ps[b] = [psctx.tile([P, 2 * CTXW], F32, tag="ctx", name="ctx")
                     for _ in range(2)]
        for i in range(NCHUNK):
            ls = slice(i * CHUNK, (i + 1) * CHUNK)
            xc = []
            for k in range(KT):
                t = xpool.tile([P, CHUNK], F32R, tag="xc", name="xc")
                nc.sync.dma_start(t[:], x_in[b, k * P:(k + 1) * P, ls])
                xc.append(t)
            for k in range(KT):
                nc.scalar.copy(x_bf[b][k][:, ls], _f32(xc[k][:]))

            E_t, vT_t = [], []
            for lm in range(LM):
                lms = slice(lm * P, (lm + 1) * P)
                # kT: [128 l, 512 (h,d)] -> E = exp
                ps = psmm.tile([P, CHUNK], F32, tag="mm", name="mm")
                for k in range(KT):
                    nc.tensor.matmul(ps[:], xc[k][:, lms],
                                     wkv_sb[k][:, 0:HIDDEN],
                                     start=(k == 0), stop=(k == KT - 1))
                e = epool.tile([P, CHUNK], BF16, tag="E", name="E")
                nc.scalar.activation(e[:], ps[:],
                                     mybir.ActivationFunctionType.Exp)
                E_t.append(e)
                # vT: [128 l, 512 (h,e)] -> bf16 with ones cols per head
                ps = psmm.tile([P, CHUNK], F32, tag="mm", name="mm")
                for k in range(KT):
                    nc.tensor.matmul(ps[:], xc[k][:, lms],
                                     wkv_sb[k][:, HIDDEN:2 * HIDDEN],
                                     start=(k == 0), stop=(k == KT - 1))
                v = epool.tile([P, HEADS * VW], BF16, tag="vT", name="vT")
                v_view = v[:].rearrange("p (h e) -> p h e", e=VW)
                nc.vector.tensor_copy(
                    v_view[:, :, 0:DH],
                    ps[:].rearrange("p (h e) -> p h e", e=DH))
                nc.vector.memset(v_view[:, :, DH:VW], 1.0)
                vT_t.append(v)

            # context accumulation into persistent PSUM, one matmul per
            # head pair (block-diag packing; off-diag blocks never read).
            # start=True resets the WHOLE psum bank, so only the first
            # pair sharing a bank may issue it (it zeroes the second
            # pair's region too); the second pair accumulates from zero.
            for lm in range(LM):
                for p in range(NPAIR):
                    reg = ctx_ps[b][p // 2][:, (p % 2) * CTXW:
                                            (p % 2 + 1) * CTXW]
                    nc.tensor.matmul(
                        reg,
                        E_t[lm][:, p * P:(p + 1) * P],
                        vT_t[lm][:, p * CTXW:(p + 1) * CTXW],
                        start=(i == 0 and lm == 0 and p % 2 == 0),
                        stop=(i == NCHUNK - 1 and lm == LM - 1),
                        skip_group_check=(p % 2 == 1))

    def finalize(b):
        # normalize ctx rows by the accumulated rowsum -> block-diag bf16
        ctxP[b] = []
        for p in range(NPAIR):
            acc = ctx_ps[b][p // 2]
            base = (p % 2) * CTXW
            r = rpool.tile([P, 1], F32, tag="recip", name="recip")
            nc.vector.reciprocal(r[0:DH, 0:1],
                                 acc[0:DH, base + DH:base + DH + 1])
            nc.vector.reciprocal(r[DH:P, 0:1],
                                 acc[DH:P, base + CTXW - 2:base + CTXW - 1])
            t = cpool.tile([P, P], BF16, tag="ctxP", name="ctxP")
            nc.vector.memset(t[:], 0.0)
            nc.vector.tensor_scalar_mul(t[0:DH, 0:DH],
                                        acc[0:DH, base:base + DH],
                                        r[0:DH, 0:1])
            nc.vector.tensor_scalar_mul(t[DH:P, DH:P],
                                        acc[DH:P, base + VW:base + VW + DH],
                                        r[DH:P, 0:1])
            ctxP[b].append(t)

    def build_m_and_pass2(b):
        # A_pair = ctxn_pair^T @ Wq_pair : [128 (h,e), 512 c]
        A_bf = []
        for p in range(NPAIR):
            ps = psmm.tile([P, DIM], F32, tag="mm", name="mm")
            nc.tensor.matmul(ps[:], ctxP[b][p][:], wq_bf[p][:],
                             start=True, stop=True)
            a = apool.tile([P, DIM], BF16, tag="A", name="A")
            nc.vector.tensor_copy(a[:], ps[:])
            A_bf.append(a)
        # M^T[c, o] = sum_pairs A_pair[he, c]^T-contract WoutT_pair[he, o]
        Mt_bf = []
        for ct in range(KT):
            ps = psmm.tile([P, DIM], F32, tag="mm", name="mm")
            for p in range(NPAIR):
                nc.tensor.matmul(ps[:], A_bf[p][:, ct * P:(ct + 1) * P],
                                 wout_bf[p][:],
                                 start=(p == 0), stop=(p == NPAIR - 1))
            m = mpool.tile([P, DIM], BF16, tag="Mt", name="Mt")
            nc.vector.tensor_copy(m[:], ps[:])
            Mt_bf.append(m)
        # Pass 2: y = M x + bias
        for i in range(NCHUNK):
            ls = slice(i * CHUNK, (i + 1) * CHUNK)
            for m in range(MT):
                ps = psmm.tile([P, CHUNK], F32, tag="mm", name="mm")
                for ct in range(KT):
                    nc.tensor.matmul(ps[:], Mt_bf[ct][:, m * P:(m + 1) * P],
                                     x_bf[b][ct][:, ls],
                                     start=(ct == 0), stop=(ct == KT - 1))
                y = ypool.tile([P, CHUNK], F32, tag="ysb", name="ysb")
                nc.vector.tensor_scalar_add(y[:], ps[:], bias_sb[:, m:m + 1])
                nc.sync.dma_start(y_out[b, m * P:(m + 1) * P, ls], y[:])

    # Issue order keeps the tensor queue dense: both k/v sweeps back-to-back
    # (finalize is vector-only and overlaps), then the M-build + y passes.
    pass1(0)
    finalize(0)
    pass1(1)
    finalize(1)
    build_m_and_pass2(0)
    build_m_and_pass2(1)


def build_module():
    nc = bacc.Bacc("TRN2", target_bir_lowering=False, debug=False,
                   num_devices=NCORES)
    x_in = nc.dram_tensor("x", [BPC, DIM, L], F32R, kind="ExternalInput")
    wkvT_in = nc.dram_tensor("w_kvT", [DIM, 2 * HIDDEN], F32R,
                             kind="ExternalInput")
    wq_in = nc.dram_tensor("w_q", [HIDDEN, DIM], F32, kind="ExternalInput")
    woutT_in = nc.dram_tensor("w_outT", [HIDDEN, DIM], F32,
                              kind="ExternalInput")
    bias_in = nc.dram_tensor("bias", [P, MT], F32, kind="ExternalInput")
    y_out = nc.dram_tensor("y", [BPC, DIM, L], F32, kind="ExternalOutput")
    with tile.TileContext(nc) as tc:
        with ExitStack() as ctx:
            build_kernel(ctx, tc, x_in, wkvT_in, wq_in, woutT_in, bias_in,
                         y_out)
    nc.compile()
    return nc


def make_in_maps(x, w_qkv, w_out, b_out):
    x = np.ascontiguousarray(x, dtype=np.float32).reshape(B, DIM, L)
    w_qkv = np.asarray(w_qkv, dtype=np.float32)
    wkvT = np.ascontiguousarray(w_qkv.T[:, HIDDEN:3 * HIDDEN])
    wq = np.ascontiguousarray(w_qkv[0:HIDDEN, :])
    woutT = np.ascontiguousarray(np.asarray(w_out, dtype=np.float32).T)
    bias = np.ascontiguousarray(
        np.asarray(b_out, dtype=np.float32).reshape(MT, P).T)
    in_maps = []
    for c in range(NCORES):
        in_maps.append({
            "x": x[c * BPC:(c + 1) * BPC],
            "w_kvT": wkvT,
            "w_q": wq,
            "w_outT": woutT,
            "bias": bias,
        })
    return in_maps


_NC_CACHE = None


def kernel(x, w_qkv, w_out, b_out, *, trace=False, trace_kwargs=None):
    """Full inputs in, full output out. Shards batch across 8 NeuronCores."""
    global _NC_CACHE
    from concourse.bass_utils import run_bass_kernel_spmd

    if _NC_CACHE is None:
        _NC_CACHE = build_module()
    nc = _NC_CACHE

    in_maps = make_in_maps(x, w_qkv, w_out, b_out)
    kw = dict(trace_kwargs or {})
    res = run_bass_kernel_spmd(nc, in_maps, list(range(NCORES)),
                               trace=trace, **kw)
    y = np.empty((B, DIM, HGT, WID), dtype=np.float32)
    for c in range(NCORES):
        y[c * BPC:(c + 1) * BPC] = res.results[c]["y"].reshape(
            BPC, DIM, HGT, WID)
    kernel.last_results = res
    return y


# revision 16
# speedup vs baseline: 1.5012x; 1.0616x over previous
# Trainium2 Bass kernel for LinearAttention (nn_LinearAttention_87686052315975).
#
# Reference computation (per batch element b of 16):
#   xf = x[b].reshape(512, 4096)                      # [c, l]
#   qkv = w_qkv @ xf; q, k, v split into 8 heads x 64 dims
#   k = softmax(k, axis=l)
#   context_h = k_h @ v_h^T                           # [64, 64]
#   out_h = context_h^T @ q_h                          # [64, l]
#   y = w_out @ concat(out_h) + b_out                 # [512, l]
#
# Key restructure vs a direct mapping: since context_h is tiny, fold it into
# the weights.  y = sum_h Wout_h ctxn_h^T Wq_h x = M x with M [512, 512]
# depending only on ctx (data-dependent) and the fixed weights.  This removes
# the q projection (q never materialized) and the per-l attention pass;
# after the k/v sweep we build M (~10k PE cycles) and do one plain matmul
# y = M x + bias.
#
# Per-batch structure (2 batches per core, data-parallel over 8 cores):
#   Pass 1 (l chunked by 512):  kT/vT computed transposed (l on partitions)
#     so the context contraction over l maps onto the PE K dim; E = exp(kT)
#     cast to bf16; vT cast to bf16 with a ones column per head appended so
#     the context matmul also accumulates rowsum(E) (softmax denominator).
#     ctx accumulates in PSUM across all 32 l-subtiles (2 head-pairs per
#     bank, block-diagonal packing).  bf16 runs the N=132 context matmuls at
#     1 cycle/row (fp32r would pay 4x at N<256).  x is also cast to a
#     resident bf16 copy for pass 2.
#   Finalize:  ctxn = ctx * (1/rowsum) into block-diag bf16 tiles.
#   Build M:   A_pair = ctxn_pair^T-contract-Wq_pair  [128, 512]
#              M^T[c, o] = sum_pairs A_pair^T-contract-WoutT_pair
#   Pass 2:    y = (M^T)^T-contract-x_bf16 + bias; DMA out.
#
# Big fp32 matmuls (k/v projection) run as float32r (1 cycle/row at N>=512).
# Everything downstream of exp runs bf16 (inputs only; PSUM accumulation is
# fp32) — well inside the 2e-2 tolerance.

import numpy as np
from contextlib import ExitStack

import concourse.bass as bass
import concourse.bacc as bacc
import concourse.mybir as mybir
import concourse.tile as tile

# ---- problem constants (hardcoded per contract) ----
B, DIM, HGT, WID = 16, 512, 64, 64
L = HGT * WID            # 4096
HEADS, DH = 8, 64
HIDDEN = HEADS * DH      # 512
NCORES = 8
BPC = B // NCORES        # 2 batches per core
P = 128
CHUNK = 512
NCHUNK = L // CHUNK      # 8
KT = DIM // P            # 4 contraction tiles over channels
MT = DIM // P            # 4 output row tiles
LM = CHUNK // P          # 4 l-subtiles per chunk
NPAIR = HEADS // 2       # 4 head pairs
VW = DH + 2              # per-head vT width: 64 v cols + 2 ones cols (even N)
CTXW = 2 * VW            # 132: one pair's context block width

F32 = mybir.dt.float32
F32R = mybir.dt.float32r
BF16 = mybir.dt.bfloat16


def _f32(ap):
    return ap.bitcast(F32)


def build_kernel(ctx: ExitStack, tc: "tile.TileContext", x_in, wkvT_in, wq_in,
                 woutT_in, bias_in, y_out):
    nc = tc.nc

    wpool = ctx.enter_context(tc.tile_pool(name="weights", bufs=1))
    xpool = ctx.enter_context(tc.tile_pool(name="xc", bufs=8))
    xbpool = ctx.enter_context(tc.tile_pool(name="xbf", bufs=8))
    epool = ctx.enter_context(tc.tile_pool(name="ev", bufs=8))
    cpool = ctx.enter_context(tc.tile_pool(name="ctxp", bufs=8))
    apool = ctx.enter_context(tc.tile_pool(name="absf", bufs=4))
    mpool = ctx.enter_context(tc.tile_pool(name="mtbf", bufs=8))
    rpool = ctx.enter_context(tc.tile_pool(name="recip", bufs=8))
    ypool = ctx.enter_context(tc.tile_pool(name="ysb", bufs=4))
    psmm = ctx.enter_context(tc.tile_pool(name="psmm", bufs=4, space="PSUM"))
    psctx = ctx.enter_context(tc.tile_pool(name="psctx", bufs=4, space="PSUM"))

    # ---- load k/v weights + bias up front; wq/wout deferred (only needed
    # by build-M, which runs after both k/v sweeps) so the first x chunk's
    # DMAs aren't queued behind 4 MB of weight traffic.
    wkv_sb = [wpool.tile([P, 2 * HIDDEN], F32R, tag=f"wkv{k}", name=f"wkv{k}")
              for k in range(KT)]
    bias_sb = wpool.tile([P, MT], F32, tag="bias", name="bias")
    nc.sync.dma_start(bias_sb[:], bias_in[:])
    wq_bf, wout_bf = [], []

    def load_late_weights():
        for k in range(KT):
            s = xpool.tile([P, DIM], F32, tag="stg", name="wq_stage")
            nc.sync.dma_start(s[:], wq_in[k * P:(k + 1) * P, :])
            t = wpool.tile([P, DIM], BF16, tag=f"wq{k}", name=f"wq{k}")
            nc.vector.tensor_copy(t[:], s[:])
            wq_bf.append(t)
        for k in range(KT):
            s = xpool.tile([P, DIM], F32, tag="stg", name="wout_stage")
            nc.sync.dma_start(s[:], woutT_in[k * P:(k + 1) * P, :])
            t = wpool.tile([P, DIM], BF16, tag=f"wout{k}", name=f"wout{k}")
            nc.vector.tensor_copy(t[:], s[:])
            wout_bf.append(t)

    x_bf = {}      # batch -> 4 resident bf16 tiles [128, 4096]
    ctxP = {}      # batch -> 4 block-diag bf16 [128, 128] normalized ctx
    ctx_ps = {}    # batch -> 2 PSUM tiles [128, 264] (2 pairs each)

    def pass1(b):
        x_bf[b] = [xbpool.tile([P, L], BF16, tag="xbf", name=f"xbf{b}_{k}")
                   for k in range(KT)]
        ctx_ps[b] = [psctx.tile([P, 2 * CTXW], F32, tag="ctx", name="ctx")
                     for _ in range(2)]
        for i in range(NCHUNK):
            ls = slice(i * CHUNK, (i + 1) * CHUNK)
            xc = []
            for k in range(KT):
                t = xpool.tile([P, CHUNK], F32R, tag="xc", name="xc")
                nc.sync.dma_start(t[:], x_in[b, k * P:(k + 1) * P, ls])
                xc.append(t)
                if b == 0 and i == 0:
                    # interleave wkv weight loads with the first x chunk so
                    # the first matmul can start after ~1.5 MB, not 3 MB
                    nc.sync.dma_start(wkv_sb[k][:],
                                      wkvT_in[k * P:(k + 1) * P, :])
            for k in range(KT):
                nc.scalar.copy(x_bf[b][k][:, ls], _f32(xc[k][:]))

            E_t, vT_t = [], []
            for lm in range(LM):
                lms = slice(lm * P, (lm + 1) * P)
                # kT: [128 l, 512 (h,d)] -> E = exp
                ps = psmm.tile([P, CHUNK], F32, tag="mm", name="mm")
                for k in range(KT):
                    nc.tensor.matmul(ps[:], xc[k][:, lms],
                                     wkv_sb[k][:, 0:HIDDEN],
                                     start=(k == 0), stop=(k == KT - 1))
                e = epool.tile([P, CHUNK], BF16, tag="E", name="E")
                nc.scalar.activation(e[:], ps[:],
                                     mybir.ActivationFunctionType.Exp)
                E_t.append(e)
                # vT: [128 l, 512 (h,e)] -> bf16 with ones cols per head
                ps = psmm.tile([P, CHUNK], F32, tag="mm", name="mm")
                for k in range(KT):
                    nc.tensor.matmul(ps[:], xc[k][:, lms],
                                     wkv_sb[k][:, HIDDEN:2 * HIDDEN],
                                     start=(k == 0), stop=(k == KT - 1))
                v = epool.tile([P, HEADS * VW], BF16, tag="vT", name="vT")
                v_view = v[:].rearrange("p (h e) -> p h e", e=VW)
                nc.vector.tensor_copy(
                    v_view[:, :, 0:DH],
                    ps[:].rearrange("p (h e) -> p h e", e=DH))
                nc.vector.memset(v_view[:, :, DH:VW], 1.0)
                vT_t.append(v)

            # context accumulation into persistent PSUM, one matmul per
            # head pair (block-diag packing; off-diag blocks never read).
            # start=True resets the WHOLE psum bank, so only the first
            # pair sharing a bank may issue it (it zeroes the second
            # pair's region too); the second pair accumulates from zero.
            for lm in range(LM):
                for p in range(NPAIR):
                    reg = ctx_ps[b][p // 2][:, (p % 2) * CTXW:
                                            (p % 2 + 1) * CTXW]
                    nc.tensor.matmul(
                        reg,
                        E_t[lm][:, p * P:(p + 1) * P],
                        vT_t[lm][:, p * CTXW:(p + 1) * CTXW],
                        start=(i == 0 and lm == 0 and p % 2 == 0),
                        stop=(i == NCHUNK - 1 and lm == LM - 1),
                        skip_group_check=(p % 2 == 1))

    def finalize(b):
        # normalize ctx rows by the accumulated rowsum -> block-diag bf16
        ctxP[b] = []
        for p in range(NPAIR):
            acc = ctx_ps[b][p // 2]
            base = (p % 2) * CTXW
            r = rpool.tile([P, 1], F32, tag="recip", name="recip")
            nc.vector.reciprocal(r[0:DH, 0:1],
                                 acc[0:DH, base + DH:base + DH + 1])
            nc.vector.reciprocal(r[DH:P, 0:1],
                                 acc[DH:P, base + CTXW - 2:base + CTXW - 1])
            t = cpool.tile([P, P], BF16, tag="ctxP", name="ctxP")
            nc.vector.memset(t[:], 0.0)
            nc.vector.tensor_scalar_mul(t[0:DH, 0:DH],
                                        acc[0:DH, base:base + DH],
                                        r[0:DH, 0:1])
            nc.vector.tensor_scalar_mul(t[DH:P, DH:P],
                                        acc[DH:P, base + VW:base + VW + DH],
                                        r[DH:P, 0:1])
            ctxP[b].append(t)

    def build_m_and_pass2(b):
        # A_pair = ctxn_pair^T @ Wq_pair : [128 (h,e), 512 c]
        # PSUM->SBUF copies split across vector/scalar so neither engine's
        # queue lags the tensor engine.
        A_bf = []
        for p in range(NPAIR):
            ps = psmm.tile([P, DIM], F32, tag="mm", name="mm")
            nc.tensor.matmul(ps[:], ctxP[b][p][:], wq_bf[p][:],
                             start=True, stop=True)
            a = apool.tile([P, DIM], BF16, tag="A", name="A")
            if p % 2 == 0:
                nc.vector.tensor_copy(a[:], ps[:])
            else:
                nc.scalar.copy(a[:], ps[:])
            A_bf.append(a)
        # M^T[c, o] = sum_pairs A_pair[he, c]^T-contract WoutT_pair[he, o]
        Mt_bf = []
        for ct in range(KT):
            ps = psmm.tile([P, DIM], F32, tag="mm", name="mm")
            for p in range(NPAIR):
                nc.tensor.matmul(ps[:], A_bf[p][:, ct * P:(ct + 1) * P],
                                 wout_bf[p][:],
                                 start=(p == 0), stop=(p == NPAIR - 1))
            m = mpool.tile([P, DIM], BF16, tag="Mt", name="Mt")
            if ct % 2 == 0:
                nc.vector.tensor_copy(m[:], ps[:])
            else:
                nc.scalar.copy(m[:], ps[:])
            Mt_bf.append(m)
        # Pass 2: y = M x + bias.  The bias-add drains PSUM; 4 adds/chunk
        # on vector alone (4.3us) would outpace tensor (3.4us/chunk), so
        # route one per chunk through scalar (activation Copy with bias).
        for i in range(NCHUNK):
            ls = slice(i * CHUNK, (i + 1) * CHUNK)
            for m in range(MT):
                ps = psmm.tile([P, CHUNK], F32, tag="mm", name="mm")
                for ct in range(KT):
                    nc.tensor.matmul(ps[:], Mt_bf[ct][:, m * P:(m + 1) * P],
                                     x_bf[b][ct][:, ls],
                                     start=(ct == 0), stop=(ct == KT - 1))
                y = ypool.tile([P, CHUNK], F32, tag="ysb", name="ysb")
                if m == MT - 1:
                    nc.scalar.add(y[:], ps[:], bias_sb[:, m:m + 1])
                else:
                    nc.vector.tensor_scalar_add(y[:], ps[:],
                                                bias_sb[:, m:m + 1])
                nc.sync.dma_start(y_out[b, m * P:(m + 1) * P, ls], y[:])

    # Issue order keeps the tensor queue dense: both k/v sweeps back-to-back
    # (finalize is vector-only and overlaps), then the M-build + y passes.
    pass1(0)
    load_late_weights()
    finalize(0)
    pass1(1)
    finalize(1)
    build_m_and_pass2(0)
    build_m_and_pass2(1)


def build_module():
    nc = bacc.Bacc("TRN2", target_bir_lowering=False, debug=False,
                   num_devices=NCORES)
    x_in = nc.dram_tensor("x", [BPC, DIM, L], F32R, kind="ExternalInput")
    wkvT_in = nc.dram_tensor("w_kvT", [DIM, 2 * HIDDEN], F32R,
                             kind="ExternalInput")
    wq_in = nc.dram_tensor("w_q", [HIDDEN, DIM], F32, kind="ExternalInput")
    woutT_in = nc.dram_tensor("w_outT", [HIDDEN, DIM], F32,
                              kind="ExternalInput")
    bias_in = nc.dram_tensor("bias", [P, MT], F32, kind="ExternalInput")
    y_out = nc.dram_tensor("y", [BPC, DIM, L], F32, kind="ExternalOutput")
    with tile.TileContext(nc) as tc:
        with ExitStack() as ctx:
            build_kernel(ctx, tc, x_in, wkvT_in, wq_in, woutT_in, bias_in,
                         y_out)
    nc.compile()
    return nc


def make_in_maps(x, w_qkv, w_out, b_out):
    x = np.ascontiguousarray(x, dtype=np.float32).reshape(B, DIM, L)
    w_qkv = np.asarray(w_qkv, dtype=np.float32)
    wkvT = np.ascontiguousarray(w_qkv.T[:, HIDDEN:3 * HIDDEN])
    wq = np.ascontiguousarray(w_qkv[0:HIDDEN, :])
    woutT = np.ascontiguousarray(np.asarray(w_out, dtype=np.float32).T)
    bias = np.ascontiguousarray(
        np.asarray(b_out, dtype=np.float32).reshape(MT, P).T)
    in_maps = []
    for c in range(NCORES):
        in_maps.append({
            "x": x[c * BPC:(c + 1) * BPC],
            "w_kvT": wkvT,
            "w_q": wq,
            "w_outT": woutT,
            "bias": bias,
        })
    return in_maps


_NC_CACHE = None


def kernel(x, w_qkv, w_out, b_out, *, trace=False, trace_kwargs=None):
    """Full inputs in, full output out. Shards batch across 8 NeuronCores."""
    global _NC_CACHE
    from concourse.bass_utils import run_bass_kernel_spmd

    if _NC_CACHE is None:
        _NC_CACHE = build_module()
    nc = _NC_CACHE

    in_maps = make_in_maps(x, w_qkv, w_out, b_out)
    kw = dict(trace_kwargs or {})
    res = run_bass_kernel_spmd(nc, in_maps, list(range(NCORES)),
                               trace=trace, **kw)
    y = np.empty((B, DIM, HGT, WID), dtype=np.float32)
    for c in range(NCORES):
        y[c * BPC:(c + 1) * BPC] = res.results[c]["y"].reshape(
            BPC, DIM, HGT, WID)
    kernel.last_results = res
    return y
